# revision 3
# baseline (speedup 1.0000x reference)
"""Biquad LPF (Direct Form 2 Transposed) on Trainium2, data-parallel over batch.

Strategy: the filter is a stable 2nd-order IIR; its impulse response decays
below fp32 precision within K taps (K chosen from the pole radius at runtime).
So y = FIR(x, h[:K]) exactly to fp32 rounding.  The FIR maps onto the
TensorEngine as banded block-Toeplitz matmuls over a chunk-transposed layout:

  - each row (T=262144) is split into 2048 chunks of 128 samples
  - SBUF holds X_T[m, q]: partition m = within-chunk sample, free q = chunk,
    with chunks ordered q=(jb,p), chunk c = 16*p + jb (so the natural
    [p, jb, m] HBM order transposes to it with one host permute)
  - "virtual blocks" of chunk data shifted by v chunks are prepended so every
    tap-block matmul is a plain contiguous column slice
  - per 512-chunk output tile: n_mm fp32 matmuls accumulate in one PSUM bank
    (lhsT_j[m, i] = h[128*j + i - m]), then PSUM->SBUF copy, then DMA out
  - host does the (pure data-movement) permutes on both sides

Sharding: batch 32 rows / 8 cores = 4 rows per core, SPMD, no collectives.
"""

import sys

sys.path.insert(0, "/opt/trn_rl_repo")

import numpy as np

# Problem shape (hardcoded per task contract).
B, T = 32, 262144
N_CORES = 8
R = B // N_CORES            # rows per core
P = 128                     # SBUF partitions == chunk length
NCHUNK = T // P             # 2048 chunks per row
NB = NCHUNK // P            # 16 blocks of 128 chunks
GTILE = 512                 # fp32 columns per PSUM bank / matmul free dim

SAMPLE_RATE = 44100.0
Q_DEFAULT = 0.707
KMAX = 6144
TAIL_TOL = 4e-6             # rel l2 of truncated tail (fp32 scan floor ~2.7e-6)


def _coeffs(freq):
    # Must match reference.biquad_lpf_coeffs exactly (python floats).
    freq = float(np.clip(freq, 20, SAMPLE_RATE / 2 - 100))
    Q = float(np.clip(Q_DEFAULT, 0.1, 30))
    w0 = 2 * np.pi * freq / SAMPLE_RATE
    sin_w0, cos_w0 = np.sin(w0), np.cos(w0)
    alpha = sin_w0 / (2 * Q)
    b0 = (1 - cos_w0) / 2
    b1 = 1 - cos_w0
    b2 = (1 - cos_w0) / 2
    a0 = 1 + alpha
    a1 = -2 * cos_w0
    a2 = 1 - alpha
    return (b0 / a0, b1 / a0, b2 / a0, a1 / a0, a2 / a0)


def _impulse(coeffs, n):
    b0, b1, b2, a1, a2 = coeffs
    h = np.zeros(n, dtype=np.float64)
    h[0] = b0
    if n > 1:
        h[1] = b1 - a1 * h[0]
    for t in range(2, n):
        h[t] = (b2 if t == 2 else 0.0) - a1 * h[t - 1] - a2 * h[t - 2]
    return h


def _plan(freq):
    """Pick tap count K (multiple of 128) so the truncated tail is below
    fp32-level, from the actual impulse response at this freq."""
    coeffs = _coeffs(freq)
    h = _impulse(coeffs, KMAX + 256)
    h2 = h * h
    total = h2.sum()
    for K in range(P, KMAX + 1, P):
        if np.sqrt(h2[K:].sum() / total) < TAIL_TOL:
            n_mm = 1 + (K - 1 + P - 1) // P
            return h[:K], K, n_mm, coeffs
    return None, None, None, coeffs


def _weights(h, K, n_mm):
    wts = np.zeros((P, P * n_mm), np.float64)
    m = np.arange(P)[:, None]
    i = np.arange(P)[None, :]
    for j in range(n_mm):
        k = P * j + i - m
        wts[:, P * j:P * (j + 1)] = np.where(
            (k >= 0) & (k < K), h[np.clip(k, 0, K - 1)], 0.0
        )
    return np.ascontiguousarray(wts.astype(np.float32))


def _xt_col_chunks(V):
    """Chunk index feeding each XTbuf column (negative => zeros)."""
    W = P * V + NCHUNK
    w = np.arange(W)
    idx = np.empty(W, np.int64)
    vpart = w < P * V
    v = V - (w // P)
    p = w % P
    idx[vpart] = (NB * p - v)[vpart]
    q = w - P * V
    idx[~vpart] = (NB * (q % P) + q // P)[~vpart]
    return idx, W


def _build_xt(x_rows, V, col_idx):
    """x_rows (R, T) -> XTbuf (R, 128, W) fp32."""
    valid = col_idx >= 0
    ci = np.clip(col_idx, 0, NCHUNK - 1)
    chunks = x_rows.reshape(x_rows.shape[0], NCHUNK, P)
    xt = chunks[:, ci, :] * valid[None, :, None]
    return np.ascontiguousarray(xt.transpose(0, 2, 1))


def _unscramble(yt):
    """yt (R, 128, NCHUNK) in [i, (jb, p)] order -> y natural (R, T)."""
    y3 = yt.reshape(yt.shape[0], P, NB, P)          # [r, i, jb, p]
    return np.ascontiguousarray(y3.transpose(0, 3, 2, 1)).reshape(yt.shape[0], T)


_BUILT = {}
LAST = None  # BassKernelResults of the most recent device run (for test.py)


def _get_program(n_mm):
    if n_mm in _BUILT:
        return _BUILT[n_mm]
    from concourse import bacc
    import concourse.tile as tile
    import concourse.mybir as mybir

    V = n_mm - 1
    W = P * V + NCHUNK
    nc = bacc.Bacc(
        "TRN2", target_bir_lowering=False, debug=False, num_devices=N_CORES
    )
    xt_d = nc.dram_tensor("xt", [R, P, W], mybir.dt.float32, kind="ExternalInput")
    w_d = nc.dram_tensor("wts", [P, P * n_mm], mybir.dt.float32, kind="ExternalInput")
    yt_d = nc.dram_tensor("yt", [R, P, NCHUNK], mybir.dt.float32, kind="ExternalOutput")

    with tile.TileContext(nc) as tc:
        with (
            tc.tile_pool(name="wp", bufs=1) as wp,
            tc.tile_pool(name="xp", bufs=2) as xp,
            tc.tile_pool(name="yp", bufs=2) as yp,
            tc.tile_pool(name="pp", bufs=2, space="PSUM") as pp,
        ):
            wt = wp.tile([P, P * n_mm], mybir.dt.float32)
            nc.sync.dma_start(wt[:], w_d[:, :])
            for r in range(R):
                xt = xp.tile([P, W], mybir.dt.float32, tag="xrow")
                nc.sync.dma_start(xt[:], xt_d[r])
                ps = pp.tile([P, NCHUNK], mybir.dt.float32, tag="ps")
                yt = yp.tile([P, NCHUNK], mybir.dt.float32, tag="yrow")
                for g in range(0, NCHUNK, GTILE):
                    for j in range(n_mm):
                        c0 = P * V + g - P * j
                        nc.tensor.matmul(
                            ps[:, g:g + GTILE],
                            wt[:, P * j:P * (j + 1)],
                            xt[:, c0:c0 + GTILE],
                            start=(j == 0),
                            stop=(j == n_mm - 1),
                        )
                    # PSUM->SBUF evacuation, alternating engines per bank.
                    if (g // GTILE) % 2 == 0:
                        nc.vector.tensor_copy(yt[:, g:g + GTILE], ps[:, g:g + GTILE])
                    else:
                        nc.scalar.copy(yt[:, g:g + GTILE], ps[:, g:g + GTILE])
                nc.sync.dma_start(yt_d[r], yt[:])
    nc.compile()
    _BUILT[n_mm] = nc
    return nc


def _host_scan(x, coeffs):
    # Exact fallback for pathological (extremely low) cutoffs: vectorized
    # over rows, fp32 like the reference scan.
    b0, b1, b2, a1, a2 = [np.float32(c) for c in coeffs]
    y = np.empty_like(x)
    w1 = np.zeros(x.shape[0], np.float32)
    w2 = np.zeros(x.shape[0], np.float32)
    for t in range(x.shape[1]):
        xt = x[:, t]
        yt = b0 * xt + w1
        w1 = b1 * xt - a1 * yt + w2
        w2 = b2 * xt - a2 * yt
        y[:, t] = yt
    return y


def kernel(x, freq):
    x = np.ascontiguousarray(np.asarray(x, dtype=np.float32))
    assert x.shape == (B, T), x.shape
    h, K, n_mm, coeffs = _plan(float(np.asarray(freq)))
    if h is None:
        return _host_scan(x, coeffs)

    V = n_mm - 1
    wts = _weights(h, K, n_mm)
    col_idx, W = _xt_col_chunks(V)

    nc = _get_program(n_mm)
    from concourse.bass_utils import run_bass_kernel_spmd

    in_maps = [
        {"xt": _build_xt(x[c * R:(c + 1) * R], V, col_idx), "wts": wts}
        for c in range(N_CORES)
    ]
    res = run_bass_kernel_spmd(nc, in_maps, core_ids=list(range(N_CORES)))
    global LAST
    LAST = res
    y = np.empty((B, T), np.float32)
    for c in range(N_CORES):
        y[c * R:(c + 1) * R] = _unscramble(res.results[c]["yt"])
    return y


# revision 23
# speedup vs baseline: 1.8199x; 1.8199x over previous
"""Biquad LPF (Direct Form 2 Transposed) on Trainium2, data-parallel over batch.

Strategy: the filter is a stable 2nd-order IIR; its impulse response decays
below fp32 precision within K taps (K chosen from the pole radius at runtime).
So y = FIR(x, h[:K]) exactly to fp32 rounding.  The FIR maps onto the
TensorEngine as banded block-Toeplitz matmuls over a chunk-transposed layout:

  - each row (T=262144) is split into 2048 chunks of 128 samples
  - SBUF holds X_T[m, q]: partition m = within-chunk sample, free q = chunk,
    with chunks ordered q=(jb,p), chunk c = 16*p + jb (so the natural
    [p, jb, m] HBM order transposes to it with one host permute)
  - "virtual blocks" of chunk data shifted by v chunks are prepended so every
    tap-block matmul is a plain contiguous column slice
  - per 512-chunk output tile: n_mm fp32 matmuls accumulate in one PSUM bank
    (lhsT_j[m, i] = h[128*j + i - m]), then PSUM->SBUF copy, then DMA out
  - host does the (pure data-movement) permutes on both sides

Sharding: batch 32 rows / 8 cores = 4 rows per core, SPMD, no collectives.
"""

import sys

sys.path.insert(0, "/opt/trn_rl_repo")

import numpy as np

# Problem shape (hardcoded per task contract).
B, T = 32, 262144
N_CORES = 8
R = B // N_CORES            # rows per core
P = 128                     # SBUF partitions == chunk length
NCHUNK = T // P             # 2048 chunks per row
NB = NCHUNK // P            # 16 blocks of 128 chunks
GTILE = 512                 # fp32 columns per PSUM bank / matmul free dim

SAMPLE_RATE = 44100.0
Q_DEFAULT = 0.707
KMAX = 6144
TAIL_TOL = 4e-6             # rel l2 of truncated tail (fp32 scan floor ~2.7e-6)


def _coeffs(freq):
    # Must match reference.biquad_lpf_coeffs exactly (python floats).
    freq = float(np.clip(freq, 20, SAMPLE_RATE / 2 - 100))
    Q = float(np.clip(Q_DEFAULT, 0.1, 30))
    w0 = 2 * np.pi * freq / SAMPLE_RATE
    sin_w0, cos_w0 = np.sin(w0), np.cos(w0)
    alpha = sin_w0 / (2 * Q)
    b0 = (1 - cos_w0) / 2
    b1 = 1 - cos_w0
    b2 = (1 - cos_w0) / 2
    a0 = 1 + alpha
    a1 = -2 * cos_w0
    a2 = 1 - alpha
    return (b0 / a0, b1 / a0, b2 / a0, a1 / a0, a2 / a0)


def _impulse(coeffs, n):
    b0, b1, b2, a1, a2 = coeffs
    h = np.zeros(n, dtype=np.float64)
    h[0] = b0
    if n > 1:
        h[1] = b1 - a1 * h[0]
    for t in range(2, n):
        h[t] = (b2 if t == 2 else 0.0) - a1 * h[t - 1] - a2 * h[t - 2]
    return h


def _plan(freq):
    """Pick tap count K (multiple of 128) so the truncated tail is below
    fp32-level, from the actual impulse response at this freq."""
    coeffs = _coeffs(freq)
    h = _impulse(coeffs, KMAX + 256)
    h2 = h * h
    total = h2.sum()
    for K in range(P, KMAX + 1, P):
        if np.sqrt(h2[K:].sum() / total) < TAIL_TOL:
            n_mm = 1 + (K - 1 + P - 1) // P
            return h[:K], K, n_mm, coeffs
    return None, None, None, coeffs


def _weights(h, K, n_mm):
    wts = np.zeros((P, P * n_mm), np.float64)
    m = np.arange(P)[:, None]
    i = np.arange(P)[None, :]
    for j in range(n_mm):
        k = P * j + i - m
        wts[:, P * j:P * (j + 1)] = np.where(
            (k >= 0) & (k < K), h[np.clip(k, 0, K - 1)], 0.0
        )
    return np.ascontiguousarray(wts.astype(np.float32))


def _xt_col_chunks(V):
    """Chunk index feeding each XTbuf column (negative => zeros)."""
    W = P * V + NCHUNK
    w = np.arange(W)
    idx = np.empty(W, np.int64)
    vpart = w < P * V
    v = V - (w // P)
    p = w % P
    idx[vpart] = (NB * p - v)[vpart]
    q = w - P * V
    idx[~vpart] = (NB * (q % P) + q // P)[~vpart]
    return idx, W


def _build_xt(x_rows, V, col_idx):
    """x_rows (R, T) -> XTbuf (R, 128, W) fp32."""
    valid = col_idx >= 0
    ci = np.clip(col_idx, 0, NCHUNK - 1)
    chunks = x_rows.reshape(x_rows.shape[0], NCHUNK, P)
    xt = chunks[:, ci, :] * valid[None, :, None]
    return np.ascontiguousarray(xt.transpose(0, 2, 1))


def _unscramble(yt):
    """yt (R, 128, NCHUNK) in [i, (jb, p)] order -> y natural (R, T)."""
    y3 = yt.reshape(yt.shape[0], P, NB, P)          # [r, i, jb, p]
    return np.ascontiguousarray(y3.transpose(0, 3, 2, 1)).reshape(yt.shape[0], T)


_BUILT = {}
LAST = None  # BassKernelResults of the most recent device run (for test.py)

# Matmul operand dtype: float32 is exact (lowers to 2 half-rate HW passes,
# ~38us PE-busy); float32r streams at full rate (~12us PE-busy) with
# tf32-like internal rounding (~1.4e-4 rel err measured on HW).
MM_DTYPE = "float32r"
# g-tiles (512 chunks) per DMA piece; piece carries a P*V halo.
G_PER_PIECE = 2
FIRST_G = 0          # leading small piece per row (0 = uniform pieces)
XBUFS = 8
YBUFS = 8
PBUFS = 4
COPY_ENG = "v"       # "alt" = alternate DVE/ACT per bank; "v" = DVE only
                     # ("v" keeps the ACT queue free to issue stores promptly)
STORE_PER_G = False


def _get_program(n_mm, mm_dtype=None):
    """Build the SPMD Bass program.

    mm_dtype "float32"/"float32r": direct matmuls on fp32 data.
    mm_dtype "split16"/"splitbf": x and wts are split hi/lo into two fp16/bf16
    planes; each (tap, g-tile) accumulates Whi@Xhi + Whi@Xlo + Wlo@Xhi in
    PSUM (the dropped Wlo@Xlo term is ~2^-20 relative).
    """
    mm_dtype = mm_dtype or MM_DTYPE
    key = (n_mm, mm_dtype)
    if key in _BUILT:
        return _BUILT[key]
    from concourse import bacc
    import concourse.tile as tile
    import concourse.mybir as mybir

    split = mm_dtype in ("split16", "splitbf")
    if split:
        dt_mm = mybir.dt.float16 if mm_dtype == "split16" else mybir.dt.bfloat16
        NPLANE = 2
    else:
        dt_mm = getattr(mybir.dt, mm_dtype)
        NPLANE = 1
    V = n_mm - 1
    W = P * V + NCHUNK
    # Per-row piece layout in g-tiles (FIRST_G leads so compute starts early).
    gs = []
    rem = NCHUNK // GTILE
    for g in ([FIRST_G] if FIRST_G else []) + [G_PER_PIECE] * 99:
        g = min(g, rem)
        if g <= 0:
            break
        gs.append(g)
        rem -= g

    nc = bacc.Bacc(
        "TRN2", target_bir_lowering=False, debug=False, num_devices=N_CORES
    )
    # With planes: xt[r, p, 0, :] = hi plane, xt[r, p, 1, :] = lo plane.
    xt_d = nc.dram_tensor("xt", [R, P, NPLANE, W], dt_mm, kind="ExternalInput")
    w_d = nc.dram_tensor("wts", [P, NPLANE, P * n_mm], dt_mm, kind="ExternalInput")
    yt_d = nc.dram_tensor("yt", [R, P, NCHUNK], mybir.dt.float32, kind="ExternalOutput")

    with tile.TileContext(nc) as tc:
        with (
            tc.tile_pool(name="wp", bufs=1) as wp,
            tc.tile_pool(name="xp", bufs=XBUFS) as xp,
            tc.tile_pool(name="yp", bufs=YBUFS) as yp,
            tc.tile_pool(name="pp", bufs=PBUFS, space="PSUM") as pp,
        ):
            wt = wp.tile([P, NPLANE, P * n_mm], dt_mm)
            # Weights go on the ACT ring: the SP ring is FIFO and piece 1's
            # load must complete first so compute starts early.
            nc.scalar.dma_start(wt[:], w_d[:, :, :])
            # (weight plane, x plane) accumulation terms; lo*lo is dropped.
            terms = [(0, 0), (0, 1), (1, 0)] if split else [(0, 0)]
            cnt = 0
            for r in range(R):
                c_base = 0
                for ng in gs:
                    piece = ng * GTILE
                    wp_cols = piece + P * V
                    xt = xp.tile([P, NPLANE, wp_cols], dt_mm, tag="xpiece")
                    nc.sync.dma_start(
                        xt[:], xt_d[r, :, :, c_base:c_base + wp_cols]
                    )
                    ps = pp.tile([P, piece], mybir.dt.float32, tag="ps")
                    yt = yp.tile([P, piece], mybir.dt.float32, tag="ypiece")
                    # Weight-major order: one weight load amortized over the
                    # whole piece (tap-inner order churns LDWEIGHTS).
                    for j in range(n_mm):
                        for ti, (wpl, xpl) in enumerate(terms):
                            for gi in range(ng):
                                c0 = gi * GTILE + P * (V - j)
                                nc.tensor.matmul(
                                    ps[:, gi * GTILE:(gi + 1) * GTILE],
                                    wt[:, wpl, P * j:P * (j + 1)],
                                    xt[:, xpl, c0:c0 + GTILE],
                                    start=(j == 0 and ti == 0),
                                    stop=(j == n_mm - 1 and ti == len(terms) - 1),
                                    skip_group_check=True,
                                )
                    for gi in range(ng):
                        dst = yt[:, gi * GTILE:(gi + 1) * GTILE]
                        src = ps[:, gi * GTILE:(gi + 1) * GTILE]
                        if COPY_ENG == "alt":
                            eng = nc.vector if cnt % 2 == 0 else nc.scalar
                        else:
                            eng = nc.vector
                        (eng.tensor_copy if eng is nc.vector else eng.copy)(dst, src)
                        cnt += 1
                    # Stores on the ACT ring (loads own the SP ring's FIFO).
                    if STORE_PER_G:
                        for gi in range(ng):
                            nc.scalar.dma_start(
                                yt_d[r, :, c_base + gi * GTILE:c_base + (gi + 1) * GTILE],
                                yt[:, gi * GTILE:(gi + 1) * GTILE],
                            )
                    else:
                        nc.scalar.dma_start(
                            yt_d[r, :, c_base:c_base + piece], yt[:]
                        )
                    c_base += piece
    nc.compile()
    _BUILT[key] = nc
    return nc


def _host_scan(x, coeffs):
    # Exact fallback for pathological (extremely low) cutoffs: vectorized
    # over rows, fp32 like the reference scan.
    b0, b1, b2, a1, a2 = [np.float32(c) for c in coeffs]
    y = np.empty_like(x)
    w1 = np.zeros(x.shape[0], np.float32)
    w2 = np.zeros(x.shape[0], np.float32)
    for t in range(x.shape[1]):
        xt = x[:, t]
        yt = b0 * xt + w1
        w1 = b1 * xt - a1 * yt + w2
        w2 = b2 * xt - a2 * yt
        y[:, t] = yt
    return y


def _split_planes(a, dtype16):
    """a (fp32, [..., W]) -> hi/lo planes stacked on axis -2, shape
    [..., 2, W]: a ~= hi + lo with each plane exactly representable."""
    hi = a.astype(dtype16)
    lo = (a - hi.astype(np.float32)).astype(dtype16)
    return np.ascontiguousarray(np.stack([hi, lo], axis=-2))


def kernel(x, freq):
    x = np.ascontiguousarray(np.asarray(x, dtype=np.float32))
    assert x.shape == (B, T), x.shape
    h, K, n_mm, coeffs = _plan(float(np.asarray(freq)))
    if h is None:
        return _host_scan(x, coeffs)

    V = n_mm - 1
    wts = _weights(h, K, n_mm)
    col_idx, W = _xt_col_chunks(V)

    nc = _get_program(n_mm)
    from concourse.bass_utils import run_bass_kernel_spmd

    split = MM_DTYPE in ("split16", "splitbf")
    if split:
        import ml_dtypes
        dtype16 = np.float16 if MM_DTYPE == "split16" else ml_dtypes.bfloat16
        wts_dev = _split_planes(wts, dtype16)                      # [P, 2, n_mm*P]
        in_maps = [
            {
                "xt": _split_planes(
                    _build_xt(x[c * R:(c + 1) * R], V, col_idx), dtype16
                ),                                                 # [R, P, 2, W]
                "wts": wts_dev,
            }
            for c in range(N_CORES)
        ]
    else:
        in_maps = [
            {
                "xt": _build_xt(x[c * R:(c + 1) * R], V, col_idx)[:, :, None, :],
                "wts": wts[:, None, :],
            }
            for c in range(N_CORES)
        ]
    res = run_bass_kernel_spmd(nc, in_maps, core_ids=list(range(N_CORES)))
    global LAST
    LAST = res
    y = np.empty((B, T), np.float32)
    for c in range(N_CORES):
        y[c * R:(c + 1) * R] = _unscramble(res.results[c]["yt"])
    return y
